# revision 10
# baseline (speedup 1.0000x reference)
"""Trainium2 Bass kernel for nn_Energyator (moe_routing).

Computation (per species s of 8):
    x = LeakyReLU_0.1(b[s] @ W1[s] + b1[s])        # (B, 256)
    x = LeakyReLU_0.1(x @ W2[s] + b2[s])
    x = LeakyReLU_0.1(x @ W3[s] + b3[s])
    A += x @ W4[s] + b4[s]                          # (B, 1)
out = sigmoid(A)                                    # (B, 1)

Sharding: data-parallel over the batch axis (dim 1 of b) across 8
NeuronCores; per-species weights are replicated. Each core computes its
8192-sample shard fully on-chip.

Per-core kernel design:
  - activations live in SBUF in transposed layout [units(part), samples]
    (bf16), so layer biases are per-partition and PE contracts over the
    partition dim; weights are the stationary operand (lhsT = W[in, out]).
  - sample tiles of N=512 (one PSUM bank per matmul output chunk).
  - PSUM->SBUF eviction fused with bias + LeakyReLU via ScalarE
    Prelu(alpha=0.1 per-partition AP), writing bf16.
  - layer 4 (256->1 per species) accumulates into one [1, 512] PSUM bank
    across all 8 species (16 matmuls), then one sigmoid + one output DMA.
  - input tiles are host-pre-transposed to [S, F, B_shard] bf16 so the
    DMA reads 1KB-contiguous rows per partition.
"""

import os
import sys

for _p in (
    "/root/.axon_site",
    "/root/.axon_site/_ro/trn_rl_repo",
    "/root/.axon_site/_ro/pypackages",
    "/opt/trn_rl_repo",
):
    if os.path.isdir(_p) and _p not in sys.path:
        sys.path.append(_p)

import numpy as np
import ml_dtypes

import concourse.bass as bass
import concourse.mybir as mybir
import concourse.tile as tile
import concourse.bass_utils as _bu
from concourse.bass_utils import run_bass_kernel_spmd

if os.environ.get("LDWOPT", "0") == "1" and not getattr(_bu, "_ldwopt_patched", False):
    _orig_run_command = _bu.run_command

    def _run_command_ldwopt(argv, **kw):
        argv = [
            "--enable-ldw-opt=true" if a == "--enable-ldw-opt=false" else a
            for a in argv
        ]
        return _orig_run_command(argv, **kw)

    _bu.run_command = _run_command_ldwopt
    _bu._ldwopt_patched = True

BF16 = mybir.dt.bfloat16
F32 = mybir.dt.float32
AF = mybir.ActivationFunctionType

S = 8          # species
B = 65536      # full batch
F = 256        # input features
U = 256        # hidden units
NCORES = 8
BC = B // NCORES   # samples per core
N = 512            # samples per tile (one PSUM bank)
NT = BC // N       # sample tiles per core
NL = 3             # hidden layers
KC = 2             # contraction chunks (256/128)
MC = 2             # output-unit chunks (256/128)
IOBUFS = 8         # input tile ring depth

NBCOL = NL * S * MC + 2  # bias columns + [b4sum, alpha]


def _fix_waits(nc):
    """This walrus build accepts very few sync-wait commands per
    instruction. Split excess waits onto injected EventSemaphore
    instructions on the same engine: the NX stalls on each in order, so
    semantics are identical and every instruction carries <= 1 wait."""
    for bb in nc.main_func.blocks:
        insts = bb.instructions
        out = []
        changed = False
        for ins in insts:
            si = getattr(ins, "sync_info", None)
            if si is not None and len(si.on_wait) > 1 and ins.engine is not None:
                waits = list(si.on_wait)
                for w in waits[:-1]:
                    out.append(
                        mybir.InstEventSemaphore(
                            name=nc.get_next_instruction_name(),
                            engine=ins.engine,
                            bass_nofuse=True,
                            sync_info=mybir.SyncInfo(on_wait=[w], on_update=[]),
                        )
                    )
                si.on_wait = [waits[-1]]
                changed = True
            out.append(ins)
        if changed:
            bb.instructions = out
    return nc


NP = 1024          # samples per pair (2 PSUM banks per eviction)
NPAIR = BC // NP   # tile-pairs per core


def build_kernel():
    nc = bass.Bass()
    bT = nc.dram_tensor("bT", [S, F, BC], BF16, kind="ExternalInput")
    WT = nc.dram_tensor("WT", [128, NL * S * KC * U], BF16, kind="ExternalInput")
    BIAS = nc.dram_tensor("BIAS", [128, NBCOL], F32, kind="ExternalInput")
    W4T = nc.dram_tensor("W4T", [128, S * KC], BF16, kind="ExternalInput")
    out = nc.dram_tensor("out", [BC, 1], F32, kind="ExternalOutput")

    with tile.TileContext(nc) as tc:
        with (
            tc.tile_pool(name="const", bufs=1) as constp,
            tc.tile_pool(name="io", bufs=16) as iop,      # 2 pairs x 8 species
            tc.tile_pool(name="act", bufs=14) as actp,    # sliding window of x tiles
            tc.tile_pool(name="dve", bufs=3) as dvep,     # DVE eviction scratch
            tc.tile_pool(name="ps", bufs=3, space="PSUM") as psp,    # 3 x 2 banks
            tc.tile_pool(name="psA", bufs=1, space="PSUM") as psAp,  # 2 banks
        ):
            # ---- const loads. Weights split per (l, s), layer-major, so
            # the L1 weights land first and PE can start without waiting
            # for the full 3MB weight block.
            biast = constp.tile([128, NBCOL], F32)
            nc.sync.dma_start(out=biast, in_=BIAS[:, :])
            w4t = constp.tile([128, S * KC], BF16)
            nc.sync.dma_start(out=w4t, in_=W4T[:, :])
            wt = constp.tile([128, NL, S, KC, U], BF16)
            WTr = WT.rearrange("p (l s k m) -> p l s k m", l=NL, s=S, k=KC)
            for l in range(NL):
                for s in range(S):
                    nc.sync.dma_start(out=wt[:, l, s], in_=WTr[:, l, s])

            alpha = biast[:, NBCOL - 1 : NBCOL]

            # ---- priming: cover const DMA queues on ACT and PE up front
            scratch = constp.tile([128, 1], F32)
            nc.scalar.activation(out=scratch, in_=biast[:, 0:1], func=AF.Copy)
            nc.tensor.ldweights(wt[:, 0, 0, 0, 0:128])
            nc.tensor.ldweights(w4t[:, 0:1])

            # unique sigmoid output slices (no slot recycling)
            sig_all = constp.tile([1, NPAIR, NP], F32)

            xt_by_ps = {}

            def issue_loads(p):
                # one DMA per species for pair p: [128, k, 1024] bf16
                for s in range(S):
                    xt = iop.tile([128, KC, NP], BF16)
                    nc.sync.dma_start(
                        out=xt,
                        in_=bT[s].rearrange("(k p) n -> p k n", p=128)[
                            :, :, p * NP : (p + 1) * NP
                        ],
                    )
                    xt_by_ps[(p, s)] = xt

            issue_loads(0)
            if NPAIR > 1:
                issue_loads(1)

            # fraction of evictions offloaded to the (otherwise idle) DVE:
            # every DVE_MOD-th (p,l,s,m) unit goes to DVE via
            #   zb = psum + bias ; xn = max(0.1*zb, zb)
            DVE_MOD = int(os.environ.get("DVE_MOD", "3"))
            evict_i = 0
            for p in range(NPAIR):
                xcur = [xt_by_ps[(p, s)] for s in range(S)]
                for l in range(NL):
                    xnext = []
                    for s in range(S):
                        x = xcur[s]
                        xn = actp.tile([128, MC, NP], BF16)
                        for m in range(MC):
                            psum = psp.tile([128, 2, N], F32)
                            for k in range(KC):
                                for tt in range(2):
                                    nc.tensor.matmul(
                                        psum[:, tt, :],
                                        lhsT=wt[:, l, s, k, m * 128 : (m + 1) * 128],
                                        rhs=x[:, k, tt * N : (tt + 1) * N],
                                        start=(k == 0),
                                        stop=(k == KC - 1),
                                    )
                            bias_ap = biast[
                                :, l * S * MC + s * MC + m : l * S * MC + s * MC + m + 1
                            ]
                            evict_i += 1
                            if DVE_MOD > 0 and evict_i % DVE_MOD == 0:
                                zb = dvep.tile([128, NP], BF16)
                                nc.vector.tensor_scalar(
                                    out=zb,
                                    in0=psum.rearrange("q t n -> q (t n)"),
                                    scalar1=bias_ap,
                                    scalar2=None,
                                    op0=mybir.AluOpType.add,
                                )
                                nc.vector.scalar_tensor_tensor(
                                    out=xn[:, m, :],
                                    in0=zb,
                                    scalar=0.1,
                                    in1=zb,
                                    op0=mybir.AluOpType.mult,
                                    op1=mybir.AluOpType.max,
                                )
                            else:
                                nc.scalar.activation(
                                    out=xn[:, m, :],
                                    in_=psum.rearrange("q t n -> q (t n)"),
                                    func=AF.Prelu,
                                    bias=bias_ap,
                                    scale=1.0,
                                    alpha=alpha,
                                )
                        xnext.append(xn)
                    xcur = xnext
                    if l == 0 and p + 2 < NPAIR:
                        # prefetch pair p+2 (slots of pair p, released by
                        # this row's L1 matmuls)
                        issue_loads(p + 2)
                # layer 4: accumulate W4-weighted sums over species
                psA = psAp.tile([1, 2, N], F32)
                for s in range(S):
                    for k in range(KC):
                        for tt in range(2):
                            nc.tensor.matmul(
                                psA[:, tt, :],
                                lhsT=w4t[:, s * KC + k : s * KC + k + 1],
                                rhs=xcur[s][:, k, tt * N : (tt + 1) * N],
                                start=(s == 0 and k == 0),
                                stop=(s == S - 1 and k == KC - 1),
                            )
                nc.scalar.activation(
                    out=sig_all[:, p, :],
                    in_=psA.rearrange("q t n -> q (t n)"),
                    func=AF.Sigmoid,
                    bias=biast[:1, NBCOL - 2 : NBCOL - 1],
                    scale=1.0,
                )
            nc.sync.dma_start(
                out=out.rearrange("n o -> o n"),
                in_=sig_all.rearrange("o t n -> o (t n)"),
            )
    return _fix_waits(nc)


_NC_CACHE = None


def _get_nc():
    global _NC_CACHE
    if _NC_CACHE is None:
        _NC_CACHE = build_kernel()
    return _NC_CACHE


def _prep_consts(W1, b1, W2, b2, W3, b3, W4, b4):
    bf16 = ml_dtypes.bfloat16
    # WT[p, l, s, k, m] = W_l[s, k*128+p, m]
    W123 = np.stack([W1, W2, W3])  # [NL, S, 256, 256]
    WT = (
        W123.reshape(NL, S, KC, 128, U)
        .transpose(3, 0, 1, 2, 4)
        .reshape(128, NL * S * KC * U)
        .astype(bf16)
    )
    BIAS = np.zeros((128, NBCOL), dtype=np.float32)
    ball = np.stack([b1, b2, b3])  # [NL, S, 256]
    for l in range(NL):
        for s in range(S):
            for m in range(MC):
                BIAS[:, l * S * MC + s * MC + m] = ball[l, s, m * 128 : (m + 1) * 128]
    BIAS[:, NBCOL - 2] = float(b4.sum())
    BIAS[:, NBCOL - 1] = 0.1
    # W4T[p, s*2+k] = W4[s, k*128+p, 0]
    W4T = (
        W4[:, :, 0]
        .reshape(S, KC, 128)
        .transpose(2, 0, 1)
        .reshape(128, S * KC)
        .astype(bf16)
    )
    return WT, BIAS, W4T


def kernel(b, W1, b1, W2, b2, W3, b3, W4, b4):
    bf16 = ml_dtypes.bfloat16
    WT, BIAS, W4T = _prep_consts(W1, b1, W2, b2, W3, b3, W4, b4)

    b = np.asarray(b)
    in_maps = []
    for c in range(NCORES):
        bc = b[:, c * BC : (c + 1) * BC, :]          # [S, BC, F]
        bTc = np.ascontiguousarray(bc.transpose(0, 2, 1)).astype(bf16)
        in_maps.append({"bT": bTc, "WT": WT, "BIAS": BIAS, "W4T": W4T})

    nc = _get_nc()
    res = run_bass_kernel_spmd(nc, in_maps, core_ids=list(range(NCORES)))
    return np.concatenate([res.results[c]["out"] for c in range(NCORES)], axis=0)


# revision 11
# speedup vs baseline: 1.0204x; 1.0204x over previous
"""Trainium2 Bass kernel for nn_Energyator (moe_routing).

Computation (per species s of 8):
    x = LeakyReLU_0.1(b[s] @ W1[s] + b1[s])        # (B, 256)
    x = LeakyReLU_0.1(x @ W2[s] + b2[s])
    x = LeakyReLU_0.1(x @ W3[s] + b3[s])
    A += x @ W4[s] + b4[s]                          # (B, 1)
out = sigmoid(A)                                    # (B, 1)

Sharding: data-parallel over the batch axis (dim 1 of b) across 8
NeuronCores; per-species weights are replicated. Each core computes its
8192-sample shard fully on-chip.

Per-core kernel design:
  - activations live in SBUF in transposed layout [units(part), samples]
    (bf16), so layer biases are per-partition and PE contracts over the
    partition dim; weights are the stationary operand (lhsT = W[in, out]).
  - sample tiles of N=512 (one PSUM bank per matmul output chunk).
  - PSUM->SBUF eviction fused with bias + LeakyReLU via ScalarE
    Prelu(alpha=0.1 per-partition AP), writing bf16.
  - layer 4 (256->1 per species) accumulates into one [1, 512] PSUM bank
    across all 8 species (16 matmuls), then one sigmoid + one output DMA.
  - input tiles are host-pre-transposed to [S, F, B_shard] bf16 so the
    DMA reads 1KB-contiguous rows per partition.
"""

import os
import sys

for _p in (
    "/root/.axon_site",
    "/root/.axon_site/_ro/trn_rl_repo",
    "/root/.axon_site/_ro/pypackages",
    "/opt/trn_rl_repo",
):
    if os.path.isdir(_p) and _p not in sys.path:
        sys.path.append(_p)

import numpy as np
import ml_dtypes

import concourse.bass as bass
import concourse.mybir as mybir
import concourse.tile as tile
import concourse.bass_utils as _bu
from concourse.bass_utils import run_bass_kernel_spmd

if os.environ.get("LDWOPT", "0") == "1" and not getattr(_bu, "_ldwopt_patched", False):
    _orig_run_command = _bu.run_command

    def _run_command_ldwopt(argv, **kw):
        argv = [
            "--enable-ldw-opt=true" if a == "--enable-ldw-opt=false" else a
            for a in argv
        ]
        return _orig_run_command(argv, **kw)

    _bu.run_command = _run_command_ldwopt
    _bu._ldwopt_patched = True

BF16 = mybir.dt.bfloat16
F32 = mybir.dt.float32
AF = mybir.ActivationFunctionType

S = 8          # species
B = 65536      # full batch
F = 256        # input features
U = 256        # hidden units
NCORES = 8
BC = B // NCORES   # samples per core
N = 512            # samples per tile (one PSUM bank)
NT = BC // N       # sample tiles per core
NL = 3             # hidden layers
KC = 2             # contraction chunks (256/128)
MC = 2             # output-unit chunks (256/128)
IOBUFS = 8         # input tile ring depth

NBCOL = NL * S * MC + 2  # bias columns + [b4sum, alpha]


def _fix_waits(nc):
    """This walrus build accepts very few sync-wait commands per
    instruction. Split excess waits onto injected EventSemaphore
    instructions on the same engine: the NX stalls on each in order, so
    semantics are identical and every instruction carries <= 1 wait."""
    for bb in nc.main_func.blocks:
        insts = bb.instructions
        out = []
        changed = False
        for ins in insts:
            si = getattr(ins, "sync_info", None)
            if si is not None and len(si.on_wait) > 1 and ins.engine is not None:
                waits = list(si.on_wait)
                for w in waits[:-1]:
                    out.append(
                        mybir.InstEventSemaphore(
                            name=nc.get_next_instruction_name(),
                            engine=ins.engine,
                            bass_nofuse=True,
                            sync_info=mybir.SyncInfo(on_wait=[w], on_update=[]),
                        )
                    )
                si.on_wait = [waits[-1]]
                changed = True
            out.append(ins)
        if changed:
            bb.instructions = out
    return nc


NP = 1024          # samples per pair (2 PSUM banks per eviction)
NPAIR = BC // NP   # tile-pairs per core


def build_kernel():
    nc = bass.Bass()
    bT = nc.dram_tensor("bT", [S, F, BC], BF16, kind="ExternalInput")
    WT = nc.dram_tensor("WT", [128, NL * S * KC * U], BF16, kind="ExternalInput")
    BIAS = nc.dram_tensor("BIAS", [128, NBCOL], F32, kind="ExternalInput")
    W4T = nc.dram_tensor("W4T", [128, S * KC], BF16, kind="ExternalInput")
    out = nc.dram_tensor("out", [BC, 1], F32, kind="ExternalOutput")

    with tile.TileContext(nc) as tc:
        with (
            tc.tile_pool(name="const", bufs=1) as constp,
            tc.tile_pool(name="io", bufs=16) as iop,      # 2 pairs x 8 species
            tc.tile_pool(name="act", bufs=14) as actp,    # sliding window of x tiles
            tc.tile_pool(name="dve", bufs=3) as dvep,     # DVE eviction scratch
            tc.tile_pool(name="ps", bufs=3, space="PSUM") as psp,    # 3 x 2 banks
            tc.tile_pool(name="psA", bufs=1, space="PSUM") as psAp,  # 2 banks
        ):
            # ---- const loads (one DMA each) ----
            wt = constp.tile([128, NL, S, KC, U], BF16)
            nc.sync.dma_start(
                out=wt,
                in_=WT.rearrange("p (l s k m) -> p l s k m", l=NL, s=S, k=KC),
            )
            biast = constp.tile([128, NBCOL], F32)
            nc.sync.dma_start(out=biast, in_=BIAS[:, :])
            w4t = constp.tile([128, S * KC], BF16)
            nc.sync.dma_start(out=w4t, in_=W4T[:, :])

            alpha = biast[:, NBCOL - 1 : NBCOL]

            # ---- priming: cover const DMA queues on ACT and PE up front
            scratch = constp.tile([128, 1], F32)
            nc.scalar.activation(out=scratch, in_=biast[:, 0:1], func=AF.Copy)
            nc.tensor.ldweights(wt[:, 0, 0, 0, 0:128])
            nc.tensor.ldweights(w4t[:, 0:1])

            # unique sigmoid output slices (no slot recycling)
            sig_all = constp.tile([1, NPAIR, NP], F32)

            xt_by_ps = {}

            def issue_loads(p):
                # one DMA per species for pair p: [128, k, 1024] bf16
                for s in range(S):
                    xt = iop.tile([128, KC, NP], BF16)
                    nc.sync.dma_start(
                        out=xt,
                        in_=bT[s].rearrange("(k p) n -> p k n", p=128)[
                            :, :, p * NP : (p + 1) * NP
                        ],
                    )
                    xt_by_ps[(p, s)] = xt

            issue_loads(0)
            if NPAIR > 1:
                issue_loads(1)

            # fraction of evictions offloaded to the (otherwise idle) DVE:
            # every DVE_MOD-th (p,l,s,m) unit goes to DVE via
            #   zb = psum + bias ; xn = max(0.1*zb, zb)
            DVE_MOD = int(os.environ.get("DVE_MOD", "3"))
            evict_i = 0
            for p in range(NPAIR):
                xcur = [xt_by_ps[(p, s)] for s in range(S)]
                for l in range(NL):
                    xnext = []
                    for s in range(S):
                        x = xcur[s]
                        xn = actp.tile([128, MC, NP], BF16)
                        for m in range(MC):
                            psum = psp.tile([128, 2, N], F32)
                            for k in range(KC):
                                for tt in range(2):
                                    nc.tensor.matmul(
                                        psum[:, tt, :],
                                        lhsT=wt[:, l, s, k, m * 128 : (m + 1) * 128],
                                        rhs=x[:, k, tt * N : (tt + 1) * N],
                                        start=(k == 0),
                                        stop=(k == KC - 1),
                                    )
                            bias_ap = biast[
                                :, l * S * MC + s * MC + m : l * S * MC + s * MC + m + 1
                            ]
                            evict_i += 1
                            if DVE_MOD > 0 and evict_i % DVE_MOD == 0:
                                zb = dvep.tile([128, NP], BF16)
                                nc.vector.tensor_scalar(
                                    out=zb,
                                    in0=psum.rearrange("q t n -> q (t n)"),
                                    scalar1=bias_ap,
                                    scalar2=None,
                                    op0=mybir.AluOpType.add,
                                )
                                nc.vector.scalar_tensor_tensor(
                                    out=xn[:, m, :],
                                    in0=zb,
                                    scalar=0.1,
                                    in1=zb,
                                    op0=mybir.AluOpType.mult,
                                    op1=mybir.AluOpType.max,
                                )
                            else:
                                nc.scalar.activation(
                                    out=xn[:, m, :],
                                    in_=psum.rearrange("q t n -> q (t n)"),
                                    func=AF.Prelu,
                                    bias=bias_ap,
                                    scale=1.0,
                                    alpha=alpha,
                                )
                        xnext.append(xn)
                    xcur = xnext
                    if l == 0 and p + 2 < NPAIR:
                        # prefetch pair p+2 (slots of pair p, released by
                        # this row's L1 matmuls)
                        issue_loads(p + 2)
                # layer 4: accumulate W4-weighted sums over species
                psA = psAp.tile([1, 2, N], F32)
                for s in range(S):
                    for k in range(KC):
                        for tt in range(2):
                            nc.tensor.matmul(
                                psA[:, tt, :],
                                lhsT=w4t[:, s * KC + k : s * KC + k + 1],
                                rhs=xcur[s][:, k, tt * N : (tt + 1) * N],
                                start=(s == 0 and k == 0),
                                stop=(s == S - 1 and k == KC - 1),
                            )
                nc.scalar.activation(
                    out=sig_all[:, p, :],
                    in_=psA.rearrange("q t n -> q (t n)"),
                    func=AF.Sigmoid,
                    bias=biast[:1, NBCOL - 2 : NBCOL - 1],
                    scale=1.0,
                )
            nc.sync.dma_start(
                out=out.rearrange("n o -> o n"),
                in_=sig_all.rearrange("o t n -> o (t n)"),
            )
    return _fix_waits(nc)


_NC_CACHE = None


def _get_nc():
    global _NC_CACHE
    if _NC_CACHE is None:
        _NC_CACHE = build_kernel()
    return _NC_CACHE


def _prep_consts(W1, b1, W2, b2, W3, b3, W4, b4):
    bf16 = ml_dtypes.bfloat16
    # WT[p, l, s, k, m] = W_l[s, k*128+p, m]
    W123 = np.stack([W1, W2, W3])  # [NL, S, 256, 256]
    WT = (
        W123.reshape(NL, S, KC, 128, U)
        .transpose(3, 0, 1, 2, 4)
        .reshape(128, NL * S * KC * U)
        .astype(bf16)
    )
    BIAS = np.zeros((128, NBCOL), dtype=np.float32)
    ball = np.stack([b1, b2, b3])  # [NL, S, 256]
    for l in range(NL):
        for s in range(S):
            for m in range(MC):
                BIAS[:, l * S * MC + s * MC + m] = ball[l, s, m * 128 : (m + 1) * 128]
    BIAS[:, NBCOL - 2] = float(b4.sum())
    BIAS[:, NBCOL - 1] = 0.1
    # W4T[p, s*2+k] = W4[s, k*128+p, 0]
    W4T = (
        W4[:, :, 0]
        .reshape(S, KC, 128)
        .transpose(2, 0, 1)
        .reshape(128, S * KC)
        .astype(bf16)
    )
    return WT, BIAS, W4T


def kernel(b, W1, b1, W2, b2, W3, b3, W4, b4):
    bf16 = ml_dtypes.bfloat16
    WT, BIAS, W4T = _prep_consts(W1, b1, W2, b2, W3, b3, W4, b4)

    b = np.asarray(b)
    in_maps = []
    for c in range(NCORES):
        bc = b[:, c * BC : (c + 1) * BC, :]          # [S, BC, F]
        bTc = np.ascontiguousarray(bc.transpose(0, 2, 1)).astype(bf16)
        in_maps.append({"bT": bTc, "WT": WT, "BIAS": BIAS, "W4T": W4T})

    nc = _get_nc()
    res = run_bass_kernel_spmd(nc, in_maps, core_ids=list(range(NCORES)))
    return np.concatenate([res.results[c]["out"] for c in range(NCORES)], axis=0)
